# revision 1
# baseline (speedup 1.0000x reference)
"""Trainium2 Bass kernel for nn_KnowledgeFusion.

Math (b=8, H=W=32, d=o=256, n_obj=15, n=16 with appended mean-emb):
  embs_aug = concat([embs, mean(embs)])                  [b,16,256]
  mask     = rasterized boxes (rounded to PATCH_SIZE=2)  [b,16,1024] in {0,1}
  proj     = patches @ Wp                                [b,1024,256]
  inj      = embs_aug @ We                               [b,16,256]
  s[hw]    = sum_n mask[n,hw]   (>=1: image box row)
  out      = proj + (mask^T @ inj) / s[:,None]           [b,1024,256]

(The reference's (proj + m*inj) masked-mean collapses to this because
mask^2 == mask.)

Sharding: data-parallel over batch; core c computes batch c (Wp/We
replicated). Computed in the transposed orientation outT[o, hw] so Wp
(resp. inj) is the stationary matmul operand and the 1024-pixel axis
streams at N=512 per matmul:

  outT[o,hw] = Wp^T @ patchesT  +  inj^T @ maskN        maskN = mask/s

All matmuls run as float32r (single-pass fp32, ~4x the fp32 rate, fp32
PSUM accumulation). The 1/s normalization is folded into the mask so
proj and the injection accumulate in the same PSUM bank; 1/s itself is
computed exactly without any slow reciprocal: s is an integer in 1..16,
so broadcast s over 16 partitions (all-ones matmul), take the indicator
ind[n,hw] = (s == n+1), and matmul against weights 1/(n+1).

Inputs arrive via 3 DMAs (tiny loc first -- the mask chain is the
latency pole -- then a weights blob, then patchesT) because each
dma_start costs ~0.6us of sequencer dispatch; outputs leave via 2.
"""

import sys

sys.path.insert(0, "/opt/trn_rl_repo")

import numpy as np

import concourse.bass as bass
import concourse.bacc as bacc
import concourse.mybir as mybir
from concourse import tile
from concourse import bass_utils
from concourse.alu_op_type import AluOpType

B, H, W, D = 8, 32, 32, 256
NOBJ, N = 15, 16
HW = H * W
O = 256
FP = mybir.dt.float32
FR = mybir.dt.float32r
I32 = mybir.dt.int32
AF = mybir.ActivationFunctionType
AX = mybir.AxisListType

# weights blob layout (columns): Wp0 Wp1 We0 We1 eT0 eT1 (each eT chunk
# has 15 real columns + 1 spare for the on-device mean)
WB = 2 * O + 2 * O + 2 * N  # 1056


def _bcast(ap, free_dims):
    """AP with explicit free-dim [step, count] pairs (step 0 = broadcast)."""
    return bass.AP(ap.tensor, ap.offset, ap.ap[:1] + free_dims)


def build_nc(debug: bool = False):
    nc = bacc.Bacc("TRN2", target_bir_lowering=False, debug=debug, num_devices=B)

    loc = nc.dram_tensor("loc", [N, 4], I32, kind="ExternalInput")
    wb = nc.dram_tensor("wb", [128, WB], FR, kind="ExternalInput")
    pT = nc.dram_tensor("pT", [128, 2 * HW], FR, kind="ExternalInput")
    outT = nc.dram_tensor("outT", [O, HW], FP, kind="ExternalOutput")

    with tile.TileContext(nc) as tc:
        with (
            nc.allow_low_precision(reason="fp32r matmuls, fp32 PSUM accumulation"),
            tc.tile_pool(name="big", bufs=1) as big,
            tc.tile_pool(name="small", bufs=1) as small,
            tc.tile_pool(name="outp", bufs=2) as outp,
            tc.tile_pool(name="psT", bufs=4, space=bass.MemorySpace.PSUM) as psT,
            tc.tile_pool(name="pstmp", bufs=2, space=bass.MemorySpace.PSUM) as pstmp,
        ):
            # ---- loads: loc first (mask chain is the long pole)
            loc_sb = small.tile([N, 4], I32)
            nc.sync.dma_start(loc_sb[:], loc[:])
            wb_sb = big.tile([128, WB], FR)
            nc.sync.dma_start(wb_sb[:], wb[:])
            pT_sb = big.tile([128, 2 * HW], FR)
            nc.gpsimd.dma_start(pT_sb[:, 0:HW], pT[:, 0:HW])
            nc.sync.dma_start(pT_sb[:, HW : 2 * HW], pT[:, HW : 2 * HW])

            Wp_sb = [wb_sb[:, O * k : O * (k + 1)] for k in range(2)]
            We_sb = [wb_sb[:, 2 * O + O * k : 2 * O + O * (k + 1)] for k in range(2)]
            eT_sb = [wb_sb[:, 4 * O + N * k : 4 * O + N * (k + 1)] for k in range(2)]

            # mean embedding into the spare 16th column of each eT chunk
            for k in range(2):
                nc.vector.tensor_reduce(
                    eT_sb[k][:, NOBJ : NOBJ + 1], eT_sb[k][:, 0:NOBJ], AX.X, AluOpType.add
                )
                nc.vector.tensor_scalar_mul(
                    eT_sb[k][:, NOBJ : NOBJ + 1], eT_sb[k][:, NOBJ : NOBJ + 1], 1.0 / NOBJ
                )

            # ---- inj = embs_aug @ We -> [16, 256]
            psumI = pstmp.tile([N, O], FP, tag="pstmp")
            nc.tensor.matmul(psumI[:], eT_sb[0][:], We_sb[0][:], start=True, stop=False)
            nc.tensor.matmul(psumI[:], eT_sb[1][:], We_sb[1][:], start=False, stop=True)
            inj_sb = small.tile([N, O], FR)
            nc.scalar.activation(inj_sb[:], psumI[:], AF.Copy)

            # ---- boxes: round starts down / ends up to multiples of 2
            locm = small.tile([N, 4], I32)
            nc.vector.tensor_scalar(locm[:], loc_sb[:], 1, None, op0=AluOpType.bitwise_and)
            boxes_i = small.tile([N, 4], I32)
            nc.vector.tensor_tensor(boxes_i[:], loc_sb[:], locm[:], op=AluOpType.subtract)
            nc.vector.tensor_scalar_add(boxes_i[:, 2:4], boxes_i[:, 2:4], 2)
            boxes_f = small.tile([N, 4], FP)
            nc.vector.tensor_copy(boxes_f[:], boxes_i[:])

            # ---- row/col interval masks [16, 32]
            grid_i = small.tile([N, 32], I32)
            nc.gpsimd.iota(grid_i[:], pattern=[[1, 32]], base=0, channel_multiplier=0)
            grid_f = small.tile([N, 32], FP)
            nc.vector.tensor_copy(grid_f[:], grid_i[:])

            rowm = small.tile([N, 32], FP)
            colm = small.tile([N, 32], FP)
            tmp = small.tile([N, 32], FP, tag="cmp_tmp")
            nc.vector.tensor_scalar(tmp[:], grid_f[:], boxes_f[:, 2:3], None, op0=AluOpType.is_lt)
            nc.vector.scalar_tensor_tensor(
                rowm[:], grid_f[:], boxes_f[:, 0:1], tmp[:], op0=AluOpType.is_ge, op1=AluOpType.mult
            )
            tmp2 = small.tile([N, 32], FP, tag="cmp_tmp2")
            nc.vector.tensor_scalar(tmp2[:], grid_f[:], boxes_f[:, 3:4], None, op0=AluOpType.is_lt)
            nc.vector.scalar_tensor_tensor(
                colm[:], grid_f[:], boxes_f[:, 1:2], tmp2[:], op0=AluOpType.is_ge, op1=AluOpType.mult
            )

            # ---- mask [16, 1024] via one broadcast outer-product multiply
            mask_sb = small.tile([N, HW], FR)
            nc.vector.tensor_tensor(
                _bcast(mask_sb[:], [[W, H], [1, W]]),
                _bcast(rowm[:], [[1, H], [0, W]]),
                _bcast(colm[:], [[0, H], [1, W]]),
                op=AluOpType.mult,
            )

            # ---- 1/s exactly, no reciprocal over hw: s integer in 1..16
            ones1c = small.tile([N, 1], FP)
            nc.vector.memset(ones1c[:], 1.0)
            ones16 = small.tile([N, N], FR)
            nc.vector.tensor_copy(ones16[:], _bcast(ones1c[:], [[0, N]]))
            idx_i = small.tile([N, 1], I32)
            nc.gpsimd.iota(idx_i[:], pattern=[[1, 1]], base=1, channel_multiplier=1)
            kvec = small.tile([N, 1], FP)
            nc.vector.tensor_copy(kvec[:], idx_i[:])
            wn = small.tile([N, 1], FP)
            nc.vector.reciprocal(wn[:], kvec[:])
            w16 = small.tile([N, N], FR)
            nc.vector.tensor_copy(w16[:], _bcast(wn[:], [[0, N]]))

            ind_sb = small.tile([N, HW], FR)
            psumS = [pstmp.tile([N, 512], FP, tag="pstmp", name=f"psS{h}") for h in range(2)]
            for h in range(2):
                nc.tensor.matmul(
                    psumS[h][:], ones16[:], mask_sb[:, 512 * h : 512 * (h + 1)],
                    start=True, stop=True,
                )
                nc.vector.tensor_scalar(
                    ind_sb[:, 512 * h : 512 * (h + 1)], psumS[h][:], kvec[:, 0:1], None,
                    op0=AluOpType.is_equal,
                )

            recB_sb = small.tile([N, HW], FP)
            psumR = [pstmp.tile([N, 512], FP, tag="pstmp", name=f"psR{h}") for h in range(2)]
            for h in range(2):
                nc.tensor.matmul(
                    psumR[h][:], w16[:], ind_sb[:, 512 * h : 512 * (h + 1)],
                    start=True, stop=True,
                )
                nc.scalar.activation(recB_sb[:, 512 * h : 512 * (h + 1)], psumR[h][:], AF.Copy)

            # ---- maskN = mask * recB  (the /s folded into the mask)
            maskN_sb = small.tile([N, HW], FR)
            nc.vector.tensor_tensor(maskN_sb[:], mask_sb[:], recB_sb[:], op=AluOpType.mult)

            # ---- main: outT[oc*128:, :] = Wp^T @ pT + inj^T @ maskN
            for oc in range(2):
                o0 = 128 * oc
                o_sb = outp.tile([128, HW], FP, tag="osb")
                for hc in range(2):
                    h0 = 512 * hc
                    psum = psT.tile([128, 512], FP, tag="psT")
                    nc.tensor.matmul(
                        psum[:], Wp_sb[0][:, o0 : o0 + 128],
                        pT_sb[:, h0 : h0 + 512],
                        start=True, stop=False,
                    )
                    nc.tensor.matmul(
                        psum[:], Wp_sb[1][:, o0 : o0 + 128],
                        pT_sb[:, HW + h0 : HW + h0 + 512],
                        start=False, stop=False,
                    )
                    nc.tensor.matmul(
                        psum[:], inj_sb[:, o0 : o0 + 128], maskN_sb[:, h0 : h0 + 512],
                        start=False, stop=True,
                    )
                    if hc == 0:
                        nc.vector.tensor_copy(o_sb[:, h0 : h0 + 512], psum[:])
                    else:
                        nc.scalar.activation(o_sb[:, h0 : h0 + 512], psum[:], AF.Copy)
                eng = nc.sync if oc == 0 else nc.gpsimd
                eng.dma_start(outT[o0 : o0 + 128, :], o_sb[:])

    nc.compile()
    return nc


def make_in_maps(inputs):
    patches = np.asarray(inputs["patches"], dtype=np.float32)
    embs = np.asarray(inputs["embs"], dtype=np.float32)
    locations = np.asarray(inputs["locations"], dtype=np.int32)
    Wp = np.asarray(inputs["Wp"], dtype=np.float32)
    We = np.asarray(inputs["We"], dtype=np.float32)
    img_box = np.array([[0, 0, H, W]], dtype=np.int32)
    wb_common = np.zeros((128, WB), dtype=np.float32)
    wb_common[:, 0:O] = Wp[0:128]
    wb_common[:, O : 2 * O] = Wp[128:256]
    wb_common[:, 2 * O : 3 * O] = We[0:128]
    wb_common[:, 3 * O : 4 * O] = We[128:256]
    in_maps = []
    for b in range(B):
        eTb = embs[b].T  # [256, 15]
        wbb = wb_common.copy()
        wbb[:, 4 * O : 4 * O + NOBJ] = eTb[0:128]
        wbb[:, 4 * O + N : 4 * O + N + NOBJ] = eTb[128:256]
        pTb = patches[b].reshape(HW, D).T  # [256, 1024]
        pT2 = np.concatenate([pTb[0:128], pTb[128:256]], axis=1)  # [128, 2048]
        in_maps.append(
            {
                "loc": np.ascontiguousarray(np.concatenate([locations[b], img_box], 0)),
                "wb": wbb,
                "pT": np.ascontiguousarray(pT2),
            }
        )
    return in_maps


_NC = None


def _get_nc():
    global _NC
    if _NC is None:
        _NC = build_nc(debug=False)
    return _NC


def run(inputs, trace: bool = False, **kwargs):
    nc = _get_nc()
    res = bass_utils.run_bass_kernel_spmd(
        nc, make_in_maps(inputs), core_ids=list(range(B)), trace=trace, **kwargs
    )
    full = np.stack([res.results[b]["outT"].T for b in range(B)], axis=0)
    return np.ascontiguousarray(full).astype(np.float32), res


def kernel(**inputs) -> np.ndarray:
    full, _ = run(inputs, trace=False)
    return full



# revision 8
# speedup vs baseline: 1.2635x; 1.2635x over previous
"""Trainium2 Bass kernel for nn_KnowledgeFusion.

Math (b=8, H=W=32, d=o=256, n_obj=15):
  embs_aug = concat([embs, mean(embs)])                  [b,16,256]
  mask     = rasterized boxes (rounded to PATCH_SIZE=2)  [b,16,1024] in {0,1}
  proj     = patches @ Wp                                [b,1024,256]
  inj      = embs_aug @ We                               [b,16,256]
  s[hw]    = sum_n mask[n,hw]   (>=1: image box row)
  out      = proj + (mask^T @ inj) / s[:,None]           [b,1024,256]

The mean-emb row folds away: with inj_k = embs_k @ We (k<15),
  sum_{n<16} maskN[n] inj_n = sum_{k<15} (mask_k + 1/15) * recB * inj_k
since the image-box row has mask=1 everywhere, so the whole kernel is
  outT[o,hw] = Wp^T @ patchesT + inj^T @ ((mask + 1/15) * recB)
with recB = 1/s via the ACT engine's Reciprocal table (s is an exact
small integer, recip error ~1e-7).

Everything is bf16 (inputs cast on host, output upcast on host) to
halve HBM traffic -- the rel-err budget is 2e-2 and bf16 lands ~3e-3.

Scheduling shape (per core = one batch element):
  - 4 input DMAs on the two HWDGE queues (sync/scalar), tiny loc first:
    the loc -> mask -> s -> 1/s -> maskN -> inj-matmul chain is the
    critical path.
  - 8 zero-matmuls (0-valued operands) into the 4 main PSUM banks act
    both as the accumulation-group openers and as PE warm-up so the HAM
    clock-gate lifts to 2.4 GHz before the real matmuls arrive.
  - mask/s/recip/maskN pipelined in two 512-pixel halves so the ACT
    reciprocal of half 0 overlaps the matmuls of half 1.
  - output leaves as four 128KB bf16 DMAs as each PSUM bank evacuates.
"""

import sys

sys.path.insert(0, "/opt/trn_rl_repo")

import numpy as np

import concourse.bass as bass
import concourse.bacc as bacc
import concourse.mybir as mybir
from concourse import tile
from concourse import bass_utils
from concourse.alu_op_type import AluOpType

B, H, W, D = 8, 32, 32, 256
NOBJ, N = 15, 16
HW = H * W
O = 256
FP = mybir.dt.float32
BF = mybir.dt.bfloat16
I32 = mybir.dt.int32
AF = mybir.ActivationFunctionType

# weights blob columns (bf16): Wp0 Wp1 We0 We1 eT0 eT1 pad
WB = 4 * O + 2 * NOBJ + 2  # 1056


def _ap(ap, free_dims):
    """AP with explicit free-dim [step, count] pairs (step 0 = broadcast)."""
    return bass.AP(ap.tensor, ap.offset, ap.ap[:1] + free_dims)


def build_nc(debug: bool = False):
    nc = bacc.Bacc("TRN2", target_bir_lowering=False, debug=debug, num_devices=B)

    loc = nc.dram_tensor("loc", [N, 4], I32, kind="ExternalInput")
    wb = nc.dram_tensor("wb", [128, WB], BF, kind="ExternalInput")
    pT = nc.dram_tensor("pT", [128, 2 * HW], BF, kind="ExternalInput")
    outT = nc.dram_tensor("outT", [128, 2 * HW], BF, kind="ExternalOutput")

    with tile.TileContext(nc) as tc:
        with (
            nc.allow_low_precision(reason="bf16 matmuls, fp32 PSUM accumulation"),
            tc.tile_pool(name="big", bufs=1) as big,
            tc.tile_pool(name="small", bufs=1) as small,
            tc.tile_pool(name="outp", bufs=1) as outp,
            tc.tile_pool(name="psT", bufs=1, space=bass.MemorySpace.PSUM) as psT,
            tc.tile_pool(name="psS", bufs=1, space=bass.MemorySpace.PSUM) as psS,
            tc.tile_pool(name="psI", bufs=1, space=bass.MemorySpace.PSUM) as psI,
        ):
            # ---- input DMAs: loc first (its chain is the latency pole)
            loc_sb = small.tile([N, 4], I32)
            nc.sync.dma_start(loc_sb[:], loc[:])
            pT_sb = big.tile([128, 2 * HW], BF)
            nc.scalar.dma_start(pT_sb[:, 0:HW], pT[:, 0:HW])
            nc.sync.dma_start(pT_sb[:, HW : 2 * HW], pT[:, HW : 2 * HW])
            wb_sb = big.tile([128, WB], BF)
            nc.scalar.dma_start(wb_sb[:], wb[:])

            Wp_sb = [wb_sb[:, O * k : O * (k + 1)] for k in range(2)]
            We_sb = [wb_sb[:, 2 * O + O * k : 2 * O + O * (k + 1)] for k in range(2)]
            eT_sb = [
                wb_sb[:, 4 * O + NOBJ * k : 4 * O + NOBJ * (k + 1)] for k in range(2)
            ]

            # ---- constants (all off the critical path)
            zw = small.tile([1, 128], BF, name="zw")
            nc.gpsimd.memset(zw[:], 0.0)
            zx = small.tile([1, 512], BF, name="zx")
            nc.gpsimd.memset(zx[:], 0.0)
            ones16 = small.tile([N, N], BF, name="ones16")
            nc.gpsimd.memset(ones16[:], 1.0)
            grid_i = small.tile([N, 32], I32, name="grid")
            nc.gpsimd.iota(grid_i[:], pattern=[[1, 32]], base=0, channel_multiplier=0)
            grid_f = small.tile([N, 32], FP, name="gridf")
            nc.vector.tensor_copy(grid_f[:], grid_i[:])
            grid2_f = small.tile([N, 32], FP, name="grid2f")
            nc.vector.tensor_scalar(
                grid2_f[:], grid_f[:], 2.0, None, op0=AluOpType.subtract
            )

            # ---- 8 zero-matmuls: open the 4 PSUM accumulation groups and
            # keep the PE busy >3.4us so HAM unthrottles before real work.
            psum = [[psT.tile([128, 512], FP, name=f"ps{h}{oc}") for oc in range(2)]
                    for h in range(2)]
            for r in range(2):
                for h in range(2):
                    for oc in range(2):
                        nc.tensor.matmul(
                            psum[h][oc][:], zw[:], zx[:],
                            start=(r == 0), stop=False,
                        )

            # ---- boxes: round starts down / (ends+2) up via shifted grid
            boxes_i = small.tile([N, 4], I32, name="boxes_i")
            nc.vector.tensor_scalar(
                boxes_i[:], loc_sb[:], -2, None, op0=AluOpType.bitwise_and
            )
            boxes = small.tile([N, 4], FP, name="boxes")
            nc.vector.tensor_copy(boxes[:], boxes_i[:])

            # ---- row/col interval masks [16, 32] (bf16 0/1)
            rowm = small.tile([N, 32], BF, name="rowm")
            colm = small.tile([N, 32], BF, name="colm")
            tmp_y = small.tile([N, 32], FP, name="tmp_y")
            tmp_x = small.tile([N, 32], FP, name="tmp_x")
            # grid-2 < (end&-2)  ==  grid < (end&-2)+2
            nc.vector.tensor_scalar(
                tmp_y[:], grid2_f[:], boxes[:, 2:3], None, op0=AluOpType.is_lt
            )
            nc.vector.scalar_tensor_tensor(
                rowm[:], grid_f[:], boxes[:, 0:1], tmp_y[:],
                op0=AluOpType.is_ge, op1=AluOpType.mult,
            )
            nc.vector.tensor_scalar(
                tmp_x[:], grid2_f[:], boxes[:, 3:4], None, op0=AluOpType.is_lt
            )
            nc.vector.scalar_tensor_tensor(
                colm[:], grid_f[:], boxes[:, 1:2], tmp_x[:],
                op0=AluOpType.is_ge, op1=AluOpType.mult,
            )

            # ---- per-half pipeline: mask -> s -> 1/s -> maskN
            mask = small.tile([N, HW], BF, name="mask")
            recB = small.tile([N, HW], BF, name="recB")
            maskN = small.tile([N, HW], BF, name="maskN")
            psumS = [psS.tile([N, 512], FP, name=f"psS{h}") for h in range(2)]

            def mask_half(h):
                # mask[:, h*512:(h+1)*512] = rowm[:, h*16:+16] x colm  (outer)
                nc.vector.tensor_tensor(
                    _ap(mask[:, 512 * h : 512 * (h + 1)], [[W, 16], [1, W]]),
                    _ap(rowm[:, 16 * h : 16 * (h + 1)], [[1, 16], [0, W]]),
                    _ap(colm[:], [[0, 16], [1, W]]),
                    op=AluOpType.mult,
                )

            def s_half(h):
                nc.tensor.matmul(
                    psumS[h][:], ones16[:], mask[:, 512 * h : 512 * (h + 1)],
                    start=True, stop=True,
                )

            lnS = small.tile([N, HW], FP, name="lnS")

            def recip_half(h):
                # 1/s = exp(-ln(s)) on the otherwise-idle ACT engine
                # (ACT Reciprocal is rejected by bass; ln/exp share one
                # table set and s is an exact integer in [1,16]).
                nc.scalar.activation(
                    lnS[:, 512 * h : 512 * (h + 1)], psumS[h][:], AF.Ln
                )
                nc.scalar.activation(
                    recB[:, 512 * h : 512 * (h + 1)],
                    lnS[:, 512 * h : 512 * (h + 1)],
                    AF.Exp,
                    scale=-1.0,
                )

            def maskN_half(h):
                # (mask + 1/15) * recB  -- the +1/15 carries the mean-emb row
                nc.vector.scalar_tensor_tensor(
                    maskN[:, 512 * h : 512 * (h + 1)],
                    mask[:, 512 * h : 512 * (h + 1)],
                    1.0 / NOBJ,
                    recB[:, 512 * h : 512 * (h + 1)],
                    op0=AluOpType.add, op1=AluOpType.mult,
                )

            mask_half(0)
            mask_half(1)

            # ---- inj = embs @ We -> [15, 256] (no mean row needed)
            psumI = psI.tile([NOBJ, O], FP, name="psI")
            nc.tensor.matmul(psumI[:], eT_sb[0][:], We_sb[0][:], start=True, stop=False)
            nc.tensor.matmul(psumI[:], eT_sb[1][:], We_sb[1][:], start=False, stop=True)
            inj_sb = small.tile([NOBJ, O], BF, name="inj")
            nc.scalar.activation(inj_sb[:], psumI[:], AF.Copy)

            s_half(0)
            s_half(1)
            recip_half(0)
            maskN_half(0)
            recip_half(1)
            maskN_half(1)

            def proj_half(h):
                for oc in range(2):
                    for k in range(2):
                        nc.tensor.matmul(
                            psum[h][oc][:],
                            Wp_sb[k][:, 128 * oc : 128 * (oc + 1)],
                            pT_sb[:, HW * k + 512 * h : HW * k + 512 * (h + 1)],
                            start=False, stop=False,
                        )

            proj_half(0)
            proj_half(1)

            # ---- injection matmuls close each accumulation group
            def inj_half(h):
                for oc in range(2):
                    nc.tensor.matmul(
                        psum[h][oc][:],
                        inj_sb[:, 128 * oc : 128 * (oc + 1)],
                        maskN[0:NOBJ, 512 * h : 512 * (h + 1)],
                        start=False, stop=True,
                    )

            inj_half(0)
            inj_half(1)

            # ---- evacuate + store: out col layout oc*1024 + h*512
            # oc=0 evacs on DVE + DMA on sync; oc=1 on ACT + DMA on scalar
            o_sb = outp.tile([128, 2 * HW], BF, name="osb")
            for h in range(2):
                dst = o_sb[:, 512 * h : 512 * h + 512]
                nc.vector.tensor_copy(dst, psum[h][0][:])
            for h in range(2):
                dst = o_sb[:, 1024 + 512 * h : 1024 + 512 * h + 512]
                nc.scalar.activation(dst, psum[h][1][:], AF.Copy)
            nc.sync.dma_start(outT[:, 0:HW], o_sb[:, 0:HW])
            nc.scalar.dma_start(outT[:, HW : 2 * HW], o_sb[:, HW : 2 * HW])

    nc.compile()
    return nc


def make_in_maps(inputs):
    import ml_dtypes

    bf16 = ml_dtypes.bfloat16
    patches = np.asarray(inputs["patches"], dtype=np.float32)
    embs = np.asarray(inputs["embs"], dtype=np.float32)
    locations = np.asarray(inputs["locations"], dtype=np.int32)
    Wp = np.asarray(inputs["Wp"], dtype=np.float32)
    We = np.asarray(inputs["We"], dtype=np.float32)
    img_box = np.array([[0, 0, H, W]], dtype=np.int32)
    wb_common = np.zeros((128, WB), dtype=np.float32)
    wb_common[:, 0:O] = Wp[0:128]
    wb_common[:, O : 2 * O] = Wp[128:256]
    wb_common[:, 2 * O : 3 * O] = We[0:128]
    wb_common[:, 3 * O : 4 * O] = We[128:256]
    in_maps = []
    for b in range(B):
        eTb = embs[b].T  # [256, 15]
        wbb = wb_common.copy()
        wbb[:, 4 * O : 4 * O + NOBJ] = eTb[0:128]
        wbb[:, 4 * O + NOBJ : 4 * O + 2 * NOBJ] = eTb[128:256]
        pTb = patches[b].reshape(HW, D).T  # [256, 1024]
        pT2 = np.concatenate([pTb[0:128], pTb[128:256]], axis=1)  # [128, 2048]
        in_maps.append(
            {
                "loc": np.ascontiguousarray(np.concatenate([locations[b], img_box], 0)),
                "wb": np.ascontiguousarray(wbb.astype(bf16)),
                "pT": np.ascontiguousarray(pT2.astype(bf16)),
            }
        )
    return in_maps


_NC = None


def _get_nc():
    global _NC
    if _NC is None:
        _NC = build_nc(debug=False)
    return _NC


def run(inputs, trace: bool = False, **kwargs):
    nc = _get_nc()
    res = bass_utils.run_bass_kernel_spmd(
        nc, make_in_maps(inputs), core_ids=list(range(B)), trace=trace, **kwargs
    )
    outs = []
    for b in range(B):
        arr = np.asarray(res.results[b]["outT"]).astype(np.float32)  # [128, 2048]
        outs.append(np.concatenate([arr[:, 0:HW].T, arr[:, HW : 2 * HW].T], axis=1))
    full = np.stack(outs, axis=0)
    return np.ascontiguousarray(full).astype(np.float32), res


def kernel(**inputs) -> np.ndarray:
    full, _ = run(inputs, trace=False)
    return full


# revision 14
# speedup vs baseline: 1.3290x; 1.0519x over previous
"""Trainium2 Bass kernel for nn_KnowledgeFusion.

Math (b=8, H=W=32, d=o=256, n_obj=15):
  embs_aug = concat([embs, mean(embs)])                  [b,16,256]
  mask     = rasterized boxes (rounded to PATCH_SIZE=2)  [b,16,1024] in {0,1}
  proj     = patches @ Wp                                [b,1024,256]
  inj      = embs_aug @ We                               [b,16,256]
  s[hw]    = sum_n mask[n,hw]   (>=1: image box row)
  out      = proj + (mask^T @ inj) / s[:,None]           [b,1024,256]

The mean-emb row folds away: with inj_k = embs_k @ We (k<15),
  sum_{n<16} maskN[n] inj_n = sum_{k<15} (mask_k + 1/15) * recB * inj_k
since the image-box row has mask=1 everywhere, so the whole kernel is
  outT[o,hw] = Wp^T @ patchesT + inj^T @ ((mask + 1/15) * recB)

recB = 1/s computed exactly without any reciprocal: s is an integer in
1..16, so partition p of the replicated-s PSUM tile tests s == p+1
(one is_equal with a per-partition constant), and a [16,16] matmul
against weights 1/(p+1) collapses the one-hot back to 1/s. All ACT-
engine ops are plain Copy, so exactly one activation-table load fires,
off the critical path.

Everything is bf16 (inputs cast on host, output upcast on host) to
halve HBM traffic; rel-err lands ~4e-3 against the 2e-2 gate.

Per-core schedule (one batch element per core):
  sync queue:   loc (256B, heads the longest dep chain), then wb
  scalar queue: pT half 0, pT half 1
  PE order interleaves proj matmuls (gated on pT) with the mask-chain
  matmuls (gated on loc) so each of the four PSUM banks closes as early
  as possible; each bank evacuates to bf16 (DVE/ACT alternating) and
  leaves through its own output DMA immediately.
"""

import sys

sys.path.insert(0, "/opt/trn_rl_repo")

import numpy as np

import concourse.bass as bass
import concourse.bacc as bacc
import concourse.mybir as mybir
from concourse import tile
from concourse import bass_utils
from concourse.alu_op_type import AluOpType

B, H, W, D = 8, 32, 32, 256
NOBJ, N = 15, 16
HW = H * W
O = 256
FP = mybir.dt.float32
BF = mybir.dt.bfloat16
I32 = mybir.dt.int32
AF = mybir.ActivationFunctionType

# weights blob columns (bf16): Wp0 Wp1 We0 We1 eT0 eT1 pad
WB = 4 * O + 2 * NOBJ + 2  # 1056


def _ap(ap, free_dims):
    """AP with explicit free-dim [step, count] pairs (step 0 = broadcast)."""
    return bass.AP(ap.tensor, ap.offset, ap.ap[:1] + free_dims)


def build_nc(debug: bool = False):
    nc = bacc.Bacc("TRN2", target_bir_lowering=False, debug=debug, num_devices=B)

    loc = nc.dram_tensor("loc", [N, 4], I32, kind="ExternalInput")
    wb = nc.dram_tensor("wb", [128, WB], BF, kind="ExternalInput")
    pT = nc.dram_tensor("pT", [128, 2 * HW], BF, kind="ExternalInput")
    outT = nc.dram_tensor("outT", [128, 2 * HW], BF, kind="ExternalOutput")

    with tile.TileContext(nc) as tc:
        with (
            nc.allow_low_precision(reason="bf16 matmuls, fp32 PSUM accumulation"),
            tc.tile_pool(name="big", bufs=1) as big,
            tc.tile_pool(name="small", bufs=1) as small,
            tc.tile_pool(name="outp", bufs=1) as outp,
            tc.tile_pool(name="psT", bufs=1, space=bass.MemorySpace.PSUM) as psT,
            tc.tile_pool(name="psS", bufs=1, space=bass.MemorySpace.PSUM) as psS,
            # psumI and psumR1 share one bank slot (disjoint lifetimes)
            tc.tile_pool(name="psI", bufs=1, space=bass.MemorySpace.PSUM) as psI,
        ):
            # ---- input DMAs: loc first (its chain is the latency pole),
            # wb next (gates injpre), pT halves on the other HWDGE queue
            loc_sb = small.tile([N, 4], I32)
            nc.sync.dma_start(loc_sb[:], loc[:])
            wb_sb = big.tile([128, WB], BF)
            nc.sync.dma_start(wb_sb[:], wb[:])
            pT_sb = big.tile([128, 2 * HW], BF)
            nc.scalar.dma_start(pT_sb[:, 0:HW], pT[:, 0:HW])
            nc.scalar.dma_start(pT_sb[:, HW : 2 * HW], pT[:, HW : 2 * HW])

            Wp_sb = [wb_sb[:, O * k : O * (k + 1)] for k in range(2)]
            We_sb = [wb_sb[:, 2 * O + O * k : 2 * O + O * (k + 1)] for k in range(2)]
            eT_sb = [
                wb_sb[:, 4 * O + NOBJ * k : 4 * O + NOBJ * (k + 1)] for k in range(2)
            ]

            # ---- constants (all off the critical path)
            ones16 = small.tile([N, N], BF, name="ones16")
            nc.gpsimd.memset(ones16[:], 1.0)
            grid_i = small.tile([N, 32], I32, name="grid")
            nc.gpsimd.iota(grid_i[:], pattern=[[1, 32]], base=0, channel_multiplier=0)
            grid_f = small.tile([N, 32], FP, name="gridf")
            nc.vector.tensor_copy(grid_f[:], grid_i[:])
            grid2_f = small.tile([N, 32], FP, name="grid2f")
            nc.vector.tensor_scalar(
                grid2_f[:], grid_f[:], 2.0, None, op0=AluOpType.subtract
            )
            kidx = small.tile([N, 1], I32, name="kidx")
            nc.gpsimd.iota(kidx[:], pattern=[[1, 1]], base=1, channel_multiplier=1)
            kvec = small.tile([N, 1], FP, name="kvec")
            nc.vector.tensor_copy(kvec[:], kidx[:])
            wn = small.tile([N, 1], FP, name="wn")
            nc.vector.reciprocal(wn[:], kvec[:])
            w16 = small.tile([N, N], BF, name="w16")
            nc.vector.tensor_copy(w16[:], _ap(wn[:], [[0, N]]))

            # ---- boxes: round starts down; ends handled via shifted grid
            boxes_i = small.tile([N, 4], I32, name="boxes_i")
            nc.vector.tensor_scalar(
                boxes_i[:], loc_sb[:], -2, None, op0=AluOpType.bitwise_and
            )
            boxes = small.tile([N, 4], FP, name="boxes")
            nc.vector.tensor_copy(boxes[:], boxes_i[:])

            # ---- row/col interval masks [16, 32] (bf16 0/1)
            rowm = small.tile([N, 32], BF, name="rowm")
            colm = small.tile([N, 32], BF, name="colm")
            tmp_y = small.tile([N, 32], FP, name="tmp_y")
            tmp_x = small.tile([N, 32], FP, name="tmp_x")
            # grid-2 < (end&-2)  ==  grid < (end&-2)+2
            nc.vector.tensor_scalar(
                tmp_y[:], grid2_f[:], boxes[:, 2:3], None, op0=AluOpType.is_lt
            )
            nc.vector.scalar_tensor_tensor(
                rowm[:], grid_f[:], boxes[:, 0:1], tmp_y[:],
                op0=AluOpType.is_ge, op1=AluOpType.mult,
            )
            nc.vector.tensor_scalar(
                tmp_x[:], grid2_f[:], boxes[:, 3:4], None, op0=AluOpType.is_lt
            )
            nc.vector.scalar_tensor_tensor(
                colm[:], grid_f[:], boxes[:, 1:2], tmp_x[:],
                op0=AluOpType.is_ge, op1=AluOpType.mult,
            )

            # ---- per-half mask chain tiles
            mask = small.tile([N, HW], BF, name="mask")
            ind = small.tile([N, HW], BF, name="ind")
            maskN = small.tile([N, HW], BF, name="maskN")
            psumS = [psS.tile([N, 512], FP, name=f"psS{h}") for h in range(2)]
            psumI = psI.tile([NOBJ, 512], FP, tag="psi", name="psI")
            psumR = [
                psS.tile([N, 512], FP, name="psR0"),
                psI.tile([N, 512], FP, tag="psi", name="psR1"),
            ]
            psum = [[psT.tile([128, 512], FP, name=f"ps{h}{oc}") for oc in range(2)]
                    for h in range(2)]

            def mask_half(h):
                # mask[:, h*512:(h+1)*512] = rowm[:, h*16:+16] x colm  (outer)
                nc.vector.tensor_tensor(
                    _ap(mask[:, 512 * h : 512 * (h + 1)], [[W, 16], [1, W]]),
                    _ap(rowm[:, 16 * h : 16 * (h + 1)], [[1, 16], [0, W]]),
                    _ap(colm[:], [[0, 16], [1, W]]),
                    op=AluOpType.mult,
                )

            def s_half(h):  # s replicated over the 16 partitions
                nc.tensor.matmul(
                    psumS[h][:], ones16[:], mask[:, 512 * h : 512 * (h + 1)],
                    start=True, stop=True,
                )

            def iseq_half(h):  # partition p: ind = (s == p+1)
                nc.vector.tensor_scalar(
                    ind[:, 512 * h : 512 * (h + 1)], psumS[h][:], kvec[:, 0:1],
                    None, op0=AluOpType.is_equal,
                )

            def ind_mm_half(h):  # recB = w16^T @ ind = 1/s (replicated)
                nc.tensor.matmul(
                    psumR[h][:], w16[:], ind[:, 512 * h : 512 * (h + 1)],
                    start=True, stop=True,
                )

            def maskN_half(h):
                # (mask + 1/15) * recB  -- the +1/15 carries the mean-emb row
                nc.vector.scalar_tensor_tensor(
                    maskN[:, 512 * h : 512 * (h + 1)],
                    mask[:, 512 * h : 512 * (h + 1)],
                    1.0 / NOBJ,
                    psumR[h][:],
                    op0=AluOpType.add, op1=AluOpType.mult,
                )

            def proj_mm(h, oc, k, start):
                nc.tensor.matmul(
                    psum[h][oc][:],
                    Wp_sb[k][:, 128 * oc : 128 * (oc + 1)],
                    pT_sb[:, HW * k + 512 * h : HW * k + 512 * (h + 1)],
                    start=start, stop=False,
                )

            def inj_mm(h, oc):
                nc.tensor.matmul(
                    psum[h][oc][:],
                    inj_sb[:, 128 * oc : 128 * (oc + 1)],
                    maskN[0:NOBJ, 512 * h : 512 * (h + 1)],
                    start=False, stop=True,
                )

            # ---- emission order doubles as per-engine FIFO order and
            # MUST be topological: Tile tracks deps by trace order, so
            # every consumer is emitted after its producer.
            mask_half(0)
            mask_half(1)

            # inj = embs @ We (gated on wb only)
            nc.tensor.matmul(
                psumI[:, 0:O], eT_sb[0][:], We_sb[0][:], start=True, stop=False
            )
            nc.tensor.matmul(
                psumI[:, 0:O], eT_sb[1][:], We_sb[1][:], start=False, stop=True
            )
            inj_sb = small.tile([NOBJ, O], BF, name="inj")
            nc.scalar.activation(inj_sb[:], psumI[:, 0:O], AF.Copy)

            s_half(0)
            s_half(1)
            iseq_half(0)
            iseq_half(1)
            proj_mm(0, 0, 0, True)
            proj_mm(0, 0, 1, False)
            ind_mm_half(0)
            ind_mm_half(1)
            maskN_half(0)
            maskN_half(1)
            inj_mm(0, 0)
            proj_mm(0, 1, 0, True)
            proj_mm(0, 1, 1, False)
            inj_mm(0, 1)
            proj_mm(1, 0, 0, True)
            proj_mm(1, 0, 1, False)
            inj_mm(1, 0)
            proj_mm(1, 1, 0, True)
            proj_mm(1, 1, 1, False)
            inj_mm(1, 1)

            # ---- evacuate + store per bank: out col layout oc*1024 + h*512
            o_sb = outp.tile([128, 2 * HW], BF, name="osb")
            for idx, (h, oc) in enumerate([(0, 0), (0, 1), (1, 0), (1, 1)]):
                c0 = 1024 * oc + 512 * h
                dst = o_sb[:, c0 : c0 + 512]
                if idx % 2 == 0:
                    nc.vector.tensor_copy(dst, psum[h][oc][:])
                    nc.sync.dma_start(outT[:, c0 : c0 + 512], dst)
                else:
                    nc.scalar.activation(dst, psum[h][oc][:], AF.Copy)
                    nc.scalar.dma_start(outT[:, c0 : c0 + 512], dst)

    nc.compile()
    return nc


def make_in_maps(inputs):
    import ml_dtypes

    bf16 = ml_dtypes.bfloat16
    patches = np.asarray(inputs["patches"], dtype=np.float32)
    embs = np.asarray(inputs["embs"], dtype=np.float32)
    locations = np.asarray(inputs["locations"], dtype=np.int32)
    Wp = np.asarray(inputs["Wp"], dtype=np.float32)
    We = np.asarray(inputs["We"], dtype=np.float32)
    img_box = np.array([[0, 0, H, W]], dtype=np.int32)
    wb_common = np.zeros((128, WB), dtype=np.float32)
    wb_common[:, 0:O] = Wp[0:128]
    wb_common[:, O : 2 * O] = Wp[128:256]
    wb_common[:, 2 * O : 3 * O] = We[0:128]
    wb_common[:, 3 * O : 4 * O] = We[128:256]
    in_maps = []
    for b in range(B):
        eTb = embs[b].T  # [256, 15]
        wbb = wb_common.copy()
        wbb[:, 4 * O : 4 * O + NOBJ] = eTb[0:128]
        wbb[:, 4 * O + NOBJ : 4 * O + 2 * NOBJ] = eTb[128:256]
        pTb = patches[b].reshape(HW, D).T  # [256, 1024]
        pT2 = np.concatenate([pTb[0:128], pTb[128:256]], axis=1)  # [128, 2048]
        in_maps.append(
            {
                "loc": np.ascontiguousarray(np.concatenate([locations[b], img_box], 0)),
                "wb": np.ascontiguousarray(wbb.astype(bf16)),
                "pT": np.ascontiguousarray(pT2.astype(bf16)),
            }
        )
    return in_maps


_NC = None


def _get_nc():
    global _NC
    if _NC is None:
        _NC = build_nc(debug=False)
    return _NC


def run(inputs, trace: bool = False, **kwargs):
    nc = _get_nc()
    res = bass_utils.run_bass_kernel_spmd(
        nc, make_in_maps(inputs), core_ids=list(range(B)), trace=trace, **kwargs
    )
    outs = []
    for b in range(B):
        arr = np.asarray(res.results[b]["outT"]).astype(np.float32)  # [128, 2048]
        outs.append(np.concatenate([arr[:, 0:HW].T, arr[:, HW : 2 * HW].T], axis=1))
    full = np.stack(outs, axis=0)
    return np.ascontiguousarray(full).astype(np.float32), res


def kernel(**inputs) -> np.ndarray:
    full, _ = run(inputs, trace=False)
    return full
